# revision 8
# baseline (speedup 1.0000x reference)
"""Autoformer encoder layer on 8 Trainium2 NeuronCores.

Strategy: sequence-parallel over (B, L) with halo recompute — zero collectives.
Each of the 8 cores owns 512 rows of one batch (4 cores per batch element) and
computes the full layer for those rows. Attention is effectively banded: the
additive time bias -0.1*|i-j| suppresses weights beyond ~|i-j|>250 to < 1e-9
relative, so each query block attends to 5 neighboring 128-key blocks
(minimum reach 256). Each core recomputes K/V over its 1152-row key extent and
Q over its 640-row query extent; moving-average halos (+-12, twice) are covered
by the 64-row query halo.

Numerics: projections + scores + AV + FFN in bf16 (fp32 accumulate), O-proj and
moving averages in float32r, all vector/scalar math in fp32.
"""
import numpy as np
import ml_dtypes

import concourse.bass as bass
import concourse.tile as tile
from concourse import bacc, mybir
from concourse.bass import AP
from concourse.bass_utils import run_bass_kernel_spmd

F32 = mybir.dt.float32
F32R = mybir.dt.float32r
BF16 = mybir.dt.bfloat16
AF = mybir.ActivationFunctionType
ALU = mybir.AluOpType

B, L, D, H, DK, DFF = 2, 2048, 1024, 16, 64, 4096
NCORES = 8
PAD = 320              # zero padding on each side of L (host side)
CHUNK = 512            # output rows owned per core
QOFF = 64              # query-extent halo before owned rows
QEXT = 640             # query extent rows (5 blocks of 128)
KEXT = 1152            # key extent rows (9 blocks of 128)
NQB = QEXT // 128      # 5
NKB = KEXT // 128      # 9
NDELTA = 5             # key blocks per query block
EPS = 1e-5
MA_K = 25
NEG = -87.0            # masked logbias value (exp -> 1.6e-38 ~ 0)

_cache = {}


def _build_nc():
    nc = bacc.Bacc("TRN2", target_bir_lowering=False, debug=False,
                   num_devices=NCORES)
    # ---- per-core inputs ----
    d_xkT = nc.dram_tensor("xkT", [D, KEXT], BF16, kind="ExternalInput")
    d_xqb = nc.dram_tensor("xqb", [QEXT, D], F32, kind="ExternalInput")
    d_logb = nc.dram_tensor("logb", [NQB, 128, NDELTA * 128], F32, kind="ExternalInput")
    d_rmask = nc.dram_tensor("rmask", [NQB, 128, 1], F32, kind="ExternalInput")
    # ---- shared (replicated) inputs ----
    d_wqT = nc.dram_tensor("wqT", [D, D], BF16, kind="ExternalInput")
    d_wkT = nc.dram_tensor("wkT", [D, D], BF16, kind="ExternalInput")
    d_wvT = nc.dram_tensor("wvT", [D, D], BF16, kind="ExternalInput")
    d_woT = nc.dram_tensor("woT", [D, D], F32R, kind="ExternalInput")
    d_w1T = nc.dram_tensor("w1T", [D, DFF], BF16, kind="ExternalInput")
    d_w2T = nc.dram_tensor("w2T", [DFF, D], BF16, kind="ExternalInput")
    d_bq = nc.dram_tensor("bq", [D], F32, kind="ExternalInput")
    d_bk = nc.dram_tensor("bk", [D], F32, kind="ExternalInput")
    d_bvb = nc.dram_tensor("bvb", [1, D], BF16, kind="ExternalInput")
    d_b1 = nc.dram_tensor("b1", [DFF], F32, kind="ExternalInput")
    d_b2b = nc.dram_tensor("b2b", [1, D], BF16, kind="ExternalInput")
    d_g1 = nc.dram_tensor("g1", [D], F32, kind="ExternalInput")
    d_be1 = nc.dram_tensor("be1", [D], F32, kind="ExternalInput")
    d_g2 = nc.dram_tensor("g2", [D], F32, kind="ExternalInput")
    d_be2 = nc.dram_tensor("be2", [D], F32, kind="ExternalInput")
    d_onesb = nc.dram_tensor("onesb", [1, 128], BF16, kind="ExternalInput")
    d_identr = nc.dram_tensor("identr", [128, 128], F32R, kind="ExternalInput")
    d_ma1A = nc.dram_tensor("ma1A", [3, 128, 128], F32R, kind="ExternalInput")
    d_ma2A = nc.dram_tensor("ma2A", [2, 128, 128], F32R, kind="ExternalInput")

    d_y = nc.dram_tensor("y", [CHUNK, D], F32, kind="ExternalOutput")

    with tile.TileContext(nc) as tc:
        with (
            tc.tile_pool(name="res", bufs=1) as res,       # resident / tag-chained
            tc.tile_pool(name="xqp", bufs=2) as xqp,       # lazy xqb row chunks
            tc.tile_pool(name="stat", bufs=8) as stat,     # LN/softmax stats
        ):
            # ---------- constants ----------
            identr = res.tile([128, 128], F32R, tag="identr")
            nc.sync.dma_start(identr[:], d_identr[:, :])
            onesb = res.tile([1, 128], BF16, tag="onesb")
            nc.sync.dma_start(onesb[:], d_onesb[:, :])
            bq_sb = res.tile([128, 8], F32, tag="bq")
            nc.sync.dma_start(bq_sb[:], d_bq.ap().rearrange("(cb p) -> p cb", p=128))
            bk_sb = res.tile([128, 8], F32, tag="bk")
            nc.sync.dma_start(bk_sb[:], d_bk.ap().rearrange("(cb p) -> p cb", p=128))
            bvb_sb = res.tile([1, D], BF16, tag="bvb")
            nc.sync.dma_start(bvb_sb[:], d_bvb[:, :])
            b1_sb = res.tile([128, 32], F32, tag="b1")
            nc.sync.dma_start(b1_sb[:], d_b1.ap().rearrange("(fb p) -> p fb", p=128))
            b2b_sb = res.tile([1, D], BF16, tag="b2b")
            nc.sync.dma_start(b2b_sb[:], d_b2b[:, :])
            g1b = res.tile([128, D], F32, tag="g1b")
            nc.sync.dma_start(g1b[:], AP(tensor=d_g1, offset=0, ap=[[0, 128], [1, D]]))
            be1b = res.tile([128, D], F32, tag="be1b")
            nc.sync.dma_start(be1b[:], AP(tensor=d_be1, offset=0, ap=[[0, 128], [1, D]]))
            g2b = res.tile([128, D], F32, tag="g2b")
            nc.sync.dma_start(g2b[:], AP(tensor=d_g2, offset=0, ap=[[0, 128], [1, D]]))
            be2b = res.tile([128, D], F32, tag="be2b")
            nc.sync.dma_start(be2b[:], AP(tensor=d_be2, offset=0, ap=[[0, 128], [1, D]]))
            ma1A = res.tile([128, 3, 128], F32R, tag="ma1A")
            nc.sync.dma_start(ma1A[:], d_ma1A.ap().rearrange("a p m -> p a m"))
            ma2A = res.tile([128, 2, 128], F32R, tag="ma2A")
            nc.sync.dma_start(ma2A[:], d_ma2A.ap().rearrange("a p m -> p a m"))
            rmask_sb = res.tile([128, NQB, 1], F32, tag="rmask")
            nc.sync.dma_start(rmask_sb[:], d_rmask.ap().rearrange("qb p x -> p qb x"))
            eps_sb = res.tile([128, 1], F32, tag="eps")
            nc.vector.memset(eps_sb[:], EPS)

            # tag-chained big tensors (sequential lifetimes share one slot)
            # tag "A": xkb (proj) -> woT (O-proj) -> gT (FFN)
            # tag "B": logb (attn) -> x1 (ma1) -> x3m (ma2)
            # tag "C": qbf (attn) -> x2 (residual)
            # tag "D": kbf (attn) -> x2T (FFN1)
            logb_sb = res.tile([128, NQB, NDELTA * 128], F32, tag="B",
                               name="logb_sb")
            nc.sync.dma_start(logb_sb[:], d_logb.ap().rearrange("qb p x -> p qb x"))

            # ---------- phase 1: QKV projections (bf16) ----------
            xkb = res.tile([128, 8, KEXT], BF16, tag="A", name="xkb")
            for db in range(8):
                nc.sync.dma_start(
                    xkb[:, db, :], d_xkT.ap().rearrange("(db p) r -> p db r", p=128)[:, db, :])

            qbf = res.tile([128, 8, QEXT], BF16, tag="C", name="qbf")
            kbf = res.tile([128, 8, KEXT], BF16, tag="D", name="kbf")
            vaug = res.tile([128, NKB, H * 65], BF16, tag="vaug")
            va4 = vaug[:].rearrange("p kb (h c) -> p kb h c", c=65)
            nc.vector.memset(va4[:, :, :, 64:65], 1.0)

            with (
                tc.tile_pool(name="wpool", bufs=2) as wpool,
                tc.tile_pool(name="psA", bufs=4, space="PSUM") as psA,
            ):
                # Q: channel-major [ch, q] ; K: channel-major [ch, keys]
                for (wd, bias_sb, out_sb, width, nch, roff) in (
                    (d_wqT, bq_sb, qbf, QEXT, 2, PAD - QOFF),  # q rows at xk offset 256
                    (d_wkT, bk_sb, kbf, KEXT, 3, 0),
                ):
                    w_sb = wpool.tile([128, 8, D], BF16, tag="w", name="wproj")
                    nc.sync.dma_start(w_sb[:], wd.ap().rearrange("(db p) c -> p db c", p=128))
                    cw = width // nch
                    for cb in range(8):
                        for n in range(nch):
                            acc = psA.tile([128, 512], F32, tag="psA", name="accp")
                            for db in range(8):
                                nc.tensor.matmul(
                                    acc[:, 0:cw],
                                    w_sb[:, db, cb * 128:(cb + 1) * 128],
                                    xkb[:, db, roff + n * cw: roff + (n + 1) * cw],
                                    start=(db == 0), stop=(db == 7))
                            nc.scalar.activation(
                                out_sb[:, cb, n * cw:(n + 1) * cw], acc[:, 0:cw],
                                AF.Identity, bias=bias_sb[:, cb:cb + 1], scale=1.0)

                # V: row-major [keys, ch] + bias via K=1 ones matmul
                w_sb = wpool.tile([128, 8, D], BF16, tag="w", name="wv")
                nc.sync.dma_start(w_sb[:], d_wvT.ap().rearrange("(db p) c -> p db c", p=128))
                for kb in range(NKB):
                    for oc in range(2):
                        acc = psA.tile([128, 512], F32, tag="psA", name="accv")
                        for db in range(8):
                            nc.tensor.matmul(
                                acc[:],
                                xkb[:, db, kb * 128:(kb + 1) * 128],
                                w_sb[:, db, oc * 512:(oc + 1) * 512],
                                start=(db == 0), stop=False)
                        nc.tensor.matmul(
                            acc[:], onesb[:], bvb_sb[:, oc * 512:(oc + 1) * 512],
                            start=False, stop=True)
                        nc.vector.tensor_copy(
                            va4[:, kb, oc * 8:(oc + 1) * 8, 0:64],
                            acc[:].rearrange("p (h c) -> p h c", c=64))

            # ---------- phase 2+3: attention, O-proj, residual, ma1, LN1 ----------
            woT_sb = res.tile([128, 8, D], F32R, tag="A", name="woT_sb")
            nc.sync.dma_start(woT_sb[:], d_woT.ap().rearrange("(db p) c -> p db c", p=128))
            x1 = res.tile([128, NQB, D], F32R, tag="x1", name="x1")
            x2 = res.tile([128, NQB, D], F32R, tag="C", name="x2")
            x2T = res.tile([128, 8, QEXT], BF16, tag="D", name="x2T")

            with (
                tc.tile_pool(name="scp", bufs=2, space="PSUM") as scp,
                tc.tile_pool(name="avp", bufs=2, space="PSUM") as avp,
                tc.tile_pool(name="ppp", bufs=2, space="PSUM") as ppp,
                tc.tile_pool(name="att", bufs=2) as att,
            ):
                for qb in range(NQB):
                    aonr = att.tile([128, D], F32R, tag="aonr")
                    for h in range(H):
                        po = (h % 2) * 64
                        cb = h // 2
                        sc_ps = scp.tile([128, NDELTA * 128], F32, tag="sc", name="sc_ps")
                        for dl in range(NDELTA):
                            kb = qb + dl
                            nc.tensor.matmul(
                                sc_ps[:, dl * 128:(dl + 1) * 128],
                                kbf[po:po + 64, cb, kb * 128:(kb + 1) * 128],
                                qbf[po:po + 64, cb, qb * 128:(qb + 1) * 128],
                                start=True, stop=True)
                        ts = att.tile([128, NDELTA * 128], F32, tag="ts")
                        nc.vector.scalar_tensor_tensor(
                            out=ts[:], in0=sc_ps[:], scalar=0.125,
                            in1=logb_sb[:, qb, :], op0=ALU.mult, op1=ALU.add)
                        ex = att.tile([128, NDELTA * 128], BF16, tag="ex")
                        nc.scalar.activation(ex[:], ts[:], AF.Exp)
                        av_ps = avp.tile([128, 65], F32, tag="av", name="av_ps")
                        for dl in range(NDELTA):
                            nc.tensor.matmul(
                                av_ps[:],
                                ex[:, dl * 128:(dl + 1) * 128],
                                vaug[:, qb + dl, h * 65:(h + 1) * 65],
                                start=(dl == 0), stop=(dl == 4))
                        rec = stat.tile([128, 1], F32, tag="rec")
                        nc.vector.reciprocal(rec[:], av_ps[:, 64:65])
                        nc.vector.tensor_scalar_mul(
                            aonr[:, h * 64:(h + 1) * 64], av_ps[:, 0:64],
                            scalar1=rec[:])
                    # transpose to aoT (per-qb), then O-proj + residual
                    aoTq = att.tile([128, 8, 128], F32R, tag="aoTq")
                    for cb in range(8):
                        tp = avp.tile([128, 128], F32R, tag="av", name="tp_ps")
                        nc.tensor.transpose(tp[:], aonr[:, cb * 128:(cb + 1) * 128], identr[:])
                        nc.vector.tensor_copy(aoTq[:, cb, :], tp[:])
                    xq_t = xqp.tile([128, D], F32, tag="xq")
                    nc.sync.dma_start(
                        xq_t[:], d_xqb[qb * 128:(qb + 1) * 128, :])
                    for oc in range(2):
                        acc = ppp.tile([128, 512], F32, tag="pp", name="op_ps")
                        for cb in range(8):
                            nc.tensor.matmul(
                                acc[:], aoTq[:, cb, :],
                                woT_sb[:, cb, oc * 512:(oc + 1) * 512],
                                start=(cb == 0), stop=(cb == 7))
                        nc.vector.scalar_tensor_tensor(
                            out=x1[:, qb, oc * 512:(oc + 1) * 512], in0=acc[:],
                            scalar=rmask_sb[:, qb], in1=xq_t[:, oc * 512:(oc + 1) * 512],
                            op0=ALU.mult, op1=ALU.add)

                # ---------- ma1 + LN1 (+g1/be1) ----------
                for qb in range(NQB):
                    parts = [(ai, src) for (ai, src) in ((1, qb), (0, qb - 1), (2, qb + 1))
                             if 0 <= src < NQB]
                    mas = []
                    for oc in range(2):
                        ma_ps = ppp.tile([128, 512], F32, tag="pp", name="ma_ps")
                        for i, (ai, src) in enumerate(parts):
                            nc.tensor.matmul(
                                ma_ps[:], ma1A[:, ai, :],
                                x1[:, src, oc * 512:(oc + 1) * 512],
                                start=(i == 0), stop=(i == len(parts) - 1))
                        mas.append(ma_ps)
                    st = stat.tile([128, 2, 6], F32, tag="st")
                    for oc in range(2):
                        nc.vector.bn_stats(st[:, oc, :], mas[oc][:])
                    mv = stat.tile([128, 2], F32, tag="mv")
                    nc.vector.bn_aggr(mv[:], st[:])
                    sq = stat.tile([128, 1], F32, tag="sq")
                    nc.scalar.activation(sq[:], mv[:, 1:2], AF.Sqrt, bias=eps_sb[:])
                    rstd = stat.tile([128, 1], F32, tag="rstd")
                    nc.vector.reciprocal(rstd[:], sq[:])
                    nmr = stat.tile([128, 1], F32, tag="nmr")
                    nc.vector.scalar_tensor_tensor(
                        out=nmr[:], in0=mv[:, 0:1], scalar=-1.0, in1=rstd[:],
                        op0=ALU.mult, op1=ALU.mult)
                    t_sb = att.tile([128, D], F32, tag="ts", name="t1_sb")
                    for oc in range(2):
                        nc.scalar.activation(
                            t_sb[:, oc * 512:(oc + 1) * 512], mas[oc][:],
                            AF.Identity, bias=nmr[:], scale=rstd[:])
                    nc.vector.tensor_mul(x2[:, qb, :], t_sb[:], g1b[:])
                    nc.vector.tensor_add(x2[:, qb, :], x2[:, qb, :].bitcast(F32), be1b[:])
                    for cb in range(8):
                        tp = avp.tile([128, 128], F32R, tag="av", name="tp2_ps")
                        nc.tensor.transpose(
                            tp[:], x2[:, qb, cb * 128:(cb + 1) * 128], identr[:])
                        nc.vector.tensor_copy(x2T[:, cb, qb * 128:(qb + 1) * 128], tp[:])

            # ---------- phase 5: FFN1 + gelu ----------
            gT = res.tile([128, 32, QEXT], BF16, tag="A", name="gT")
            with (
                tc.tile_pool(name="w1p", bufs=3) as w1p,
                tc.tile_pool(name="h1p", bufs=3, space="PSUM") as h1p,
            ):
                for fb in range(32):
                    w1c = w1p.tile([128, 8, 128], BF16, tag="w1c")
                    nc.sync.dma_start(
                        w1c[:],
                        d_w1T.ap().rearrange("(db p) f -> p db f", p=128)[:, :, fb * 128:(fb + 1) * 128])
                    h1 = h1p.tile([128, 1024], F32, tag="h1")
                    for half in range(2):
                        for db in range(8):
                            nc.tensor.matmul(
                                h1[:, half * 512: half * 512 + 320],
                                w1c[:, db, :],
                                x2T[:, db, half * 320:(half + 1) * 320],
                                start=(db == 0), stop=(db == 7))
                    h1v = h1[:].rearrange("p (b c) -> p b c", b=2)[:, :, 0:320]
                    gv = gT[:, fb, :].rearrange("p (b c) -> p b c", c=320)
                    nc.scalar.activation(gv, h1v, AF.Gelu, bias=b1_sb[:, fb:fb + 1])

            # ---------- phase 6: FFN2 + residual + mask ----------
            x3m = res.tile([128, NQB, D], F32R, tag="B", name="x3m")
            with (
                tc.tile_pool(name="w2p", bufs=3) as w2p,
                tc.tile_pool(name="xap", bufs=5, space="PSUM") as xap,
                tc.tile_pool(name="ff2", bufs=3) as ff2,
            ):
                for oc in range(2):
                    accs = [xap.tile([128, 512], F32, tag="xa", name=f"xa{oc}_{i}")
                            for i in range(NQB)]
                    for fb in range(32):
                        w2c = w2p.tile([128, 512], BF16, tag="w2c")
                        nc.sync.dma_start(
                            w2c[:], d_w2T[fb * 128:(fb + 1) * 128, oc * 512:(oc + 1) * 512])
                        for qb in range(NQB):
                            nc.tensor.matmul(
                                accs[qb][:], gT[:, fb, qb * 128:(qb + 1) * 128], w2c[:],
                                start=(fb == 0), stop=False)
                    for qb in range(NQB):
                        nc.tensor.matmul(
                            accs[qb][:], onesb[:], b2b_sb[:, oc * 512:(oc + 1) * 512],
                            start=False, stop=True)
                        x3f = ff2.tile([128, 512], F32, tag="x3f")
                        nc.vector.tensor_add(
                            x3f[:], accs[qb][:],
                            x2[:, qb, oc * 512:(oc + 1) * 512].bitcast(F32))
                        nc.vector.tensor_scalar_mul(
                            x3m[:, qb, oc * 512:(oc + 1) * 512], x3f[:],
                            scalar1=rmask_sb[:, qb])

            # ---------- phase 7: moving_avg2 + LN2 -> output ----------
            with (
                tc.tile_pool(name="map", bufs=4, space="PSUM") as map_,
                tc.tile_pool(name="outp", bufs=2) as outp,
            ):
                for ob in range(4):
                    mas = []
                    for oc in range(2):
                        ma_ps = map_.tile([128, 512], F32, tag="ma2", name="ma2_ps")
                        nc.tensor.matmul(
                            ma_ps[:], ma2A[:, 0, :], x3m[:, ob, oc * 512:(oc + 1) * 512],
                            start=True, stop=False)
                        nc.tensor.matmul(
                            ma_ps[:], ma2A[:, 1, :], x3m[:, ob + 1, oc * 512:(oc + 1) * 512],
                            start=False, stop=True)
                        mas.append(ma_ps)
                    st = stat.tile([128, 2, 6], F32, tag="st")
                    for oc in range(2):
                        nc.vector.bn_stats(st[:, oc, :], mas[oc][:])
                    mv = stat.tile([128, 2], F32, tag="mv")
                    nc.vector.bn_aggr(mv[:], st[:])
                    sq = stat.tile([128, 1], F32, tag="sq")
                    nc.scalar.activation(sq[:], mv[:, 1:2], AF.Sqrt, bias=eps_sb[:])
                    rstd = stat.tile([128, 1], F32, tag="rstd")
                    nc.vector.reciprocal(rstd[:], sq[:])
                    nmr = stat.tile([128, 1], F32, tag="nmr")
                    nc.vector.scalar_tensor_tensor(
                        out=nmr[:], in0=mv[:, 0:1], scalar=-1.0, in1=rstd[:],
                        op0=ALU.mult, op1=ALU.mult)
                    t_sb = outp.tile([128, D], F32, tag="t2")
                    for oc in range(2):
                        nc.scalar.activation(
                            t_sb[:, oc * 512:(oc + 1) * 512], mas[oc][:],
                            AF.Identity, bias=nmr[:], scale=rstd[:])
                    u_sb = outp.tile([128, D], F32, tag="u2")
                    nc.vector.tensor_mul(u_sb[:], t_sb[:], g2b[:])
                    nc.vector.tensor_add(u_sb[:], u_sb[:], be2b[:])
                    nc.sync.dma_start(d_y[ob * 128:(ob + 1) * 128, :], u_sb[:])

    nc.compile()
    return nc


def _host_prep(inputs):
    x = np.asarray(inputs["x"], np.float32)
    bo = np.asarray(inputs["bo"], np.float32)

    xp = np.zeros((B, L + 2 * PAD, D), np.float32)
    xp[:, PAD:PAD + L] = x

    shared = {
        "wqT": np.ascontiguousarray(np.asarray(inputs["Wq"], np.float32).T).astype(ml_dtypes.bfloat16),
        "wkT": np.ascontiguousarray(np.asarray(inputs["Wk"], np.float32).T).astype(ml_dtypes.bfloat16),
        "wvT": np.ascontiguousarray(np.asarray(inputs["Wv"], np.float32).T).astype(ml_dtypes.bfloat16),
        "woT": np.ascontiguousarray(np.asarray(inputs["Wo"], np.float32).T),
        "w1T": np.ascontiguousarray(np.asarray(inputs["W1"], np.float32).T).astype(ml_dtypes.bfloat16),
        "w2T": np.ascontiguousarray(np.asarray(inputs["W2"], np.float32).T).astype(ml_dtypes.bfloat16),
        "bq": np.asarray(inputs["bq"], np.float32),
        "bk": np.asarray(inputs["bk"], np.float32),
        "bvb": np.asarray(inputs["bv"], np.float32).reshape(1, D).astype(ml_dtypes.bfloat16),
        "b1": np.asarray(inputs["b1"], np.float32),
        "b2b": np.asarray(inputs["b2"], np.float32).reshape(1, D).astype(ml_dtypes.bfloat16),
        "g1": np.asarray(inputs["g1"], np.float32),
        "be1": np.asarray(inputs["be1"], np.float32),
        "g2": np.asarray(inputs["g2"], np.float32),
        "be2": np.asarray(inputs["be2"], np.float32),
        "onesb": np.ones((1, 128), ml_dtypes.bfloat16),
        "identr": np.eye(128, dtype=np.float32),
    }
    # moving-average band matrices (lhsT layout: [in_row p, out_row m])
    p_i = np.arange(128)[:, None]
    m_i = np.arange(128)[None, :]
    ma1A = np.zeros((3, 128, 128), np.float32)
    ma1A[0] = (np.abs(m_i + 128 - p_i) <= 12) / MA_K   # prev in-block
    ma1A[1] = (np.abs(m_i - p_i) <= 12) / MA_K         # same
    ma1A[2] = (np.abs(m_i - 128 - p_i) <= 12) / MA_K   # next
    ma2A = np.zeros((2, 128, 128), np.float32)
    ma2A[0] = (np.abs(64 + m_i - p_i) <= 12) / MA_K    # same block (out offset 64)
    ma2A[1] = (np.abs(m_i - 64 - p_i) <= 12) / MA_K    # next block
    shared["ma1A"] = ma1A
    shared["ma2A"] = ma2A

    in_maps = []
    for c in range(NCORES):
        b, s = c // 4, 512 * (c % 4)
        xk = xp[b, s:s + KEXT]                              # orig rows [s-320, s+832)
        xq = xp[b, s + PAD - QOFF: s + PAD - QOFF + QEXT].copy()   # orig rows [s-64, s+576)
        qorig = s - QOFF + np.arange(QEXT)
        valid = (qorig >= 0) & (qorig < L)
        xq[valid] += bo
        rmask = valid.astype(np.float32).reshape(NQB, 128, 1)

        logb = np.full((NQB, 128, NDELTA * 128), NEG, np.float32)
        for qb in range(NQB):
            qo = s - QOFF + qb * 128 + np.arange(128)           # query orig rows
            for dl in range(NDELTA):
                ko = s - PAD + (qb + dl) * 128 + np.arange(128)  # key orig rows
                dist = np.abs(qo[None, :] - ko[:, None]).astype(np.float32)
                val = np.maximum(-0.1 * dist, NEG)
                bad = ~(((ko >= 0) & (ko < L))[:, None] & ((qo >= 0) & (qo < L))[None, :])
                val[bad] = NEG
                logb[qb, :, dl * 128:(dl + 1) * 128] = val

        m = dict(shared)
        m["xkT"] = np.ascontiguousarray(xk.T).astype(ml_dtypes.bfloat16)
        m["xqb"] = xq
        m["logb"] = logb
        m["rmask"] = rmask
        in_maps.append(m)
    return in_maps


def kernel(**inputs) -> np.ndarray:
    if "nc" not in _cache:
        _cache["nc"] = _build_nc()
    nc = _cache["nc"]
    in_maps = _host_prep(inputs)
    res = run_bass_kernel_spmd(nc, in_maps, core_ids=list(range(NCORES)))
    out = np.empty((B, L, D), np.float32)
    for c in range(NCORES):
        b, s = c // 4, 512 * (c % 4)
        out[b, s:s + 512] = res.results[c]["y"]
    return out
